# revision 2
# baseline (speedup 1.0000x reference)
"""Biaffine kernel for Trainium2, 8-core SPMD.

Math (reference):
    out[b,x,y,o] = bwn0 * sum_{i,j<=512} x1b[b,x,i] W_bil[o,i,j] x2b[b,y,j]
                 + bwn1 * (x1@W_lin[:512] [b,x,o] + x2@W_lin[512:] [b,y,o] + b_lin[o])
    with x1b/x2b = x append-ones, bwn = softmax(bw).

Decomposition used here (exact):
    out[b,x,y,o] = sum_{j<512} x2[b,y,j] * UT[b,o][j,x]      (step B, PE)
                 + D1[b,o,x]                                  (rank-1 bcast + DVE add)
                 + D2[b,y,o]                                  (per-partition scalar add)
    UT[b,o][j,x] = sum_{i<512} (bwn0*W_bil[o,i,j]) * x1[b,x,i]          (step A, PE)
    D1[b,o,x]    = sum_i x1[b,x,i]*G[i,o] + g0[o],  G = bwn0*W_bil[o,:,512] + bwn1*W_lin[:512,o]
    D2[b,y,o]    = sum_j x2[b,y,j]*V[j,o],          V = bwn0*W_bil[o,512,:] + bwn1*W_lin[512:,o]
    g0[o]        = bwn0*W_bil[o,512,512] + bwn1*b_lin[o]

Sharding: tensor-parallel over O (128 output channels -> 16 per core).
Matmuls run as float32r (fp32 storage, TF32-like PE datapath, ~1e-4 rel err).

Pipelining: the PE p-state ramps 1.2->2.4 GHz only under continuous
execution, so the emission order is software-pipelined as
A(0), [A(o2+1), B(o2)] for o2 in 0..7 -- step B of iteration o2 runs on the
PE right after step A of o2+1, hiding the PSUM->SBUF eviction latency of UT
and the W-tile DMA (prefetched 3 deep on the Activation-engine queue).
The D1 term is broadcast once per (b, o2-pair) with a K=1 matmul into PSUM
and folded together with the per-partition D2 scalar in a single fused
scalar_tensor_tensor eviction.
"""

import numpy as np

import concourse.bass as bass
import concourse.mybir as mybir
import concourse.tile as tile
from concourse.bass_utils import run_bass_kernel_spmd

B, L, D, O = 4, 256, 512, 128
N_CORES = 8
O_LOC = O // N_CORES          # 16 output channels per core
N_O2 = O_LOC // 2             # 8 o-pairs per core
F32 = mybir.dt.float32
F32R = mybir.dt.float32r
ADD = mybir.AluOpType.add


# --------------------------------------------------------------------------
# Workaround: this container's walrus build accepts only ONE sync wait per
# instruction ("Too many sync wait commands").  Tile's wait assignment can
# attach several.  Post-pass: hoist extra waits onto InstEventSemaphore
# wait-carriers inserted immediately before the instruction on the same
# engine stream (same stall point, identical semantics).
_WS_CTR = [0]


def _split_multi_waits(nc):
    for f in nc.m.functions:
        for blk in f.blocks:
            insts = blk.instructions
            new = []
            changed = False
            for inst in insts:
                si = inst.sync_info
                waits = list(si.on_wait) if (si and si.on_wait) else []
                if len(waits) > 1:
                    for w in waits[:-1]:
                        _WS_CTR[0] += 1
                        carrier = mybir.InstEventSemaphore(
                            name=f"waitsplit_{_WS_CTR[0]}", ins=[], outs=[]
                        )
                        carrier.engine = inst.engine
                        carrier.sync_info = mybir.SyncInfo(on_wait=[w], on_update=[])
                        new.append(carrier)
                    si.on_wait = [waits[-1]]
                    changed = True
                new.append(inst)
            if changed:
                blk.instructions = new


# --------------------------------------------------------------------------
def build_nc(split_waits=True, n_o2=N_O2):
    nc = bass.Bass("TRN2", target_bir_lowering=False, debug=False,
                   num_devices=N_CORES)

    WM = nc.dram_tensor("WM", [O_LOC, D, D], F32R, kind="ExternalInput").ap()
    X1T = nc.dram_tensor("X1T", [D, B * L], F32R, kind="ExternalInput").ap()
    X2T = nc.dram_tensor("X2T", [D, B * L], F32R, kind="ExternalInput").ap()
    G = nc.dram_tensor("G", [D, O_LOC], F32R, kind="ExternalInput").ap()
    V = nc.dram_tensor("V", [D, O_LOC], F32R, kind="ExternalInput").ap()
    G0 = nc.dram_tensor("G0", [O_LOC, 1], F32, kind="ExternalInput").ap()
    ONES = nc.dram_tensor("ONES", [128, 128], F32R, kind="ExternalInput").ap()
    OUT = nc.dram_tensor("OUT", [B, O_LOC, L, L], F32, kind="ExternalOutput").ap()

    with tile.TileContext(nc) as tc:
        with (
            tc.tile_pool(name="const", bufs=1) as cst,
            tc.tile_pool(name="w", bufs=3) as wpool,
            tc.tile_pool(name="ut", bufs=2) as utpool,
            tc.tile_pool(name="cs", bufs=4) as cspool,
        ):
            # ---- resident inputs -------------------------------------------------
            X1s = cst.tile([128, 4, B * L], F32R, tag="x1s")     # [i%128, it, b*256+x]
            nc.sync.dma_start(out=X1s[:], in_=X1T.rearrange("(it p) c -> p it c", p=128))
            Gs = cst.tile([128, 4, O_LOC], F32R, tag="gs")
            nc.sync.dma_start(out=Gs[:], in_=G.rearrange("(it p) o -> p it o", p=128))
            X2s = cst.tile([128, 4, B * L], F32R, tag="x2s")     # [j%128, jt, b*256+y]
            nc.sync.dma_start(out=X2s[:], in_=X2T.rearrange("(jt p) c -> p jt c", p=128))
            Vs = cst.tile([128, 4, O_LOC], F32R, tag="vs")
            nc.sync.dma_start(out=Vs[:], in_=V.rearrange("(jt p) o -> p jt o", p=128))
            g0s = cst.tile([O_LOC, 1], F32, tag="g0s")
            nc.sync.dma_start(out=g0s[:], in_=G0[:])
            onesAll = cst.tile([128, 128], F32R, tag="ones")
            nc.sync.dma_start(out=onesAll[:], in_=ONES[:])

            # persistent D-term tiles
            D1T = [cst.tile([O_LOC, L], F32R, tag=f"d1t{b}", name=f"d1t{b}") for b in range(B)]
            D2C = [cst.tile([128, 2, O_LOC], F32, tag=f"d2c{b}", name=f"d2c{b}") for b in range(B)]
            # rhs rows for the D1 broadcast: partition 32*b holds D1T[b]
            # flattened o-major, so D1F[32b, o2*512:(o2+1)*512] covers an o-pair.
            D1F = cst.tile([128, O_LOC * L], F32R, tag="d1f")

            # W prefetch: triggered from the Activation-engine queue so the
            # trigger is not serialized behind the OUT-DMA waits on sync.
            def emit_W(o2):
                Ws = wpool.tile([128, 2, 4, D], F32R, tag="ws")   # [i%128, oi, it, j]
                nc.scalar.dma_start(
                    out=Ws[:],
                    in_=WM[2 * o2:2 * o2 + 2].rearrange("oi (it p) j -> p oi it j", p=128),
                )
                return Ws

            # ---- precompute D-terms ---------------------------------------------
            with tc.tile_pool(name="psD", bufs=1, space="PSUM") as psD:
                for b in range(B):
                    pd1 = psD.tile([O_LOC, L], F32, tag="pd1")
                    for it in range(4):
                        nc.tensor.matmul(
                            pd1[:],
                            lhsT=Gs[:, it, :],
                            rhs=X1s[:, it, b * L:(b + 1) * L],
                            start=(it == 0), stop=(it == 3),
                        )
                    nc.vector.tensor_scalar_add(D1T[b][:], pd1[:], g0s[:, 0:1])
                    for o in range(O_LOC):
                        nc.sync.dma_start(
                            out=D1F[32 * b:32 * b + 1, o * L:(o + 1) * L],
                            in_=D1T[b][o:o + 1, :],
                        )
                    for yt in range(2):
                        pd2 = psD.tile([128, O_LOC], F32, tag="pd2")
                        for jt in range(4):
                            nc.tensor.matmul(
                                pd2[:],
                                lhsT=X2s[:, jt, b * L + yt * 128: b * L + (yt + 1) * 128],
                                rhs=Vs[:, jt, :],
                                start=(jt == 0), stop=(jt == 3),
                            )
                        nc.vector.tensor_copy(out=D2C[b][:, yt, :], in_=pd2[:])

            # ---- main loop over o-pairs, software-pipelined ----------------------
            ctx_psA = tc.tile_pool(name="psA", bufs=3, space="PSUM")
            ctx_psB = tc.tile_pool(name="psB", bufs=3, space="PSUM")
            ctx_psb = tc.tile_pool(name="psbc", bufs=2, space="PSUM")
            psA = ctx_psA.__enter__()
            psB = ctx_psB.__enter__()
            psBC = ctx_psb.__enter__()

            def emit_A(o2, Ws):
                # step A: UT[bp][j, (jt, oi, b2*256+x)] for this o-pair
                UT = [utpool.tile([128, 4, 2, 512], F32R, tag=f"utp{bp}", name=f"utp{bp}")
                      for bp in range(2)]
                for oi in range(2):
                    for jt in range(4):
                        for bp in range(2):
                            pa = psA.tile([128, 512], F32, tag="pa")
                            for it in range(4):
                                nc.tensor.matmul(
                                    pa[:],
                                    lhsT=Ws[:, oi, it, jt * 128:(jt + 1) * 128],
                                    rhs=X1s[:, it, bp * 512:(bp + 1) * 512],
                                    start=(it == 0), stop=(it == 3),
                                )
                            nc.vector.tensor_copy(
                                out=UT[bp][:, jt, oi, :], in_=pa[:])
                return UT

            def emit_B(o2, UT):
                # step B: out[y, (oi, x)] per (b, yt)
                for b in range(B):
                    # D1 broadcast for this (b, o-pair): rank-1 ones x D1row
                    pbc = psBC.tile([128, 512], F32, tag="pbc")
                    nc.tensor.matmul(
                        pbc[:],
                        lhsT=onesAll[32 * b:32 * b + 1, 0:128],
                        rhs=D1F[32 * b:32 * b + 1, o2 * 512:(o2 + 1) * 512],
                        start=True, stop=True,
                        tile_position=(32 * b, 0),
                    )
                    bp, b2 = divmod(b, 2)
                    for yt in range(2):
                        pb = psB.tile([128, 512], F32, tag="pb")
                        for jt in range(4):
                            nc.tensor.matmul(
                                pb[:],
                                lhsT=X2s[:, jt, b * L + yt * 128: b * L + (yt + 1) * 128],
                                rhs=UT[bp][:, jt, :, b2 * 256:(b2 + 1) * 256],
                                start=(jt == 0), stop=(jt == 3),
                            )
                        cs = cspool.tile([128, 512], F32, tag="cs")
                        for oi in range(2):
                            og = 2 * o2 + oi
                            # cs = (pb + D2[y,o]) + D1bc[o,x], single DVE op
                            nc.vector.scalar_tensor_tensor(
                                out=cs[:, oi * 256:(oi + 1) * 256],
                                in0=pb[:, oi * 256:(oi + 1) * 256],
                                scalar=D2C[b][:, yt, og:og + 1],
                                in1=pbc[:, oi * 256:(oi + 1) * 256],
                                op0=ADD, op1=ADD,
                            )
                            nc.sync.dma_start(
                                out=OUT[b, og, yt * 128:(yt + 1) * 128, :],
                                in_=cs[:, oi * 256:(oi + 1) * 256],
                            )

            WsQ = [emit_W(i) for i in range(min(3, n_o2))]
            UT_prev = emit_A(0, WsQ[0])
            for o2 in range(n_o2):
                if o2 + 3 < n_o2:
                    WsQ.append(emit_W(o2 + 3))
                UT_next = emit_A(o2 + 1, WsQ[o2 + 1]) if o2 + 1 < n_o2 else None
                emit_B(o2, UT_prev)
                UT_prev = UT_next

            ctx_psb.__exit__(None, None, None)
            ctx_psB.__exit__(None, None, None)
            ctx_psA.__exit__(None, None, None)

    if split_waits:
        _split_multi_waits(nc)
    return nc


_NC_CACHE = None


def _get_nc():
    global _NC_CACHE
    if _NC_CACHE is None:
        _NC_CACHE = build_nc()
    return _NC_CACHE


def _prep_inputs(x1, x2, bw, W_bil, W_lin, b_lin):
    """Host-side glue: softmax of the 2-vector, per-core slicing/layout."""
    x1 = np.asarray(x1, np.float32)
    x2 = np.asarray(x2, np.float32)
    bw = np.asarray(bw, np.float64)
    W_bil = np.asarray(W_bil, np.float32)
    W_lin = np.asarray(W_lin, np.float32)
    b_lin = np.asarray(b_lin, np.float32)

    e = np.exp(bw - bw.max())
    bwn = (e / e.sum()).astype(np.float32)
    bwn0, bwn1 = float(bwn[0]), float(bwn[1])

    x1T = np.ascontiguousarray(x1.transpose(2, 0, 1).reshape(D, B * L))
    x2T = np.ascontiguousarray(x2.transpose(2, 0, 1).reshape(D, B * L))
    ones = np.ones((128, 128), np.float32)

    in_maps = []
    for c in range(N_CORES):
        o_sl = slice(c * O_LOC, (c + 1) * O_LOC)
        Wb = W_bil[o_sl]                                   # [16, 513, 513]
        WM = np.ascontiguousarray(bwn0 * Wb[:, :D, :D])
        G = np.ascontiguousarray(bwn0 * Wb[:, :D, D].T + bwn1 * W_lin[:D, o_sl])
        V = np.ascontiguousarray(bwn0 * Wb[:, D, :D].T + bwn1 * W_lin[D:, o_sl])
        G0 = (bwn0 * Wb[:, D, D] + bwn1 * b_lin[o_sl]).reshape(O_LOC, 1)
        in_maps.append({
            "WM": WM, "X1T": x1T, "X2T": x2T,
            "G": G.astype(np.float32), "V": V.astype(np.float32),
            "G0": np.ascontiguousarray(G0, dtype=np.float32), "ONES": ones,
        })
    return in_maps


def _assemble(results):
    out = np.empty((B, L, L, O), np.float32)
    for c in range(N_CORES):
        # per-core OUT is [b, o_local, y, x] -> full is [b, x, y, o]
        out[:, :, :, c * O_LOC:(c + 1) * O_LOC] = \
            results[c]["OUT"].transpose(0, 3, 2, 1)
    return out


def kernel(**inputs):
    in_maps = _prep_inputs(**inputs)
    nc = _get_nc()
    res = run_bass_kernel_spmd(nc, in_maps, list(range(N_CORES)))
    return _assemble(res.results)


# revision 7
# speedup vs baseline: 1.0845x; 1.0845x over previous
"""Biaffine kernel for Trainium2, 8-core SPMD.

Math (reference):
    out[b,x,y,o] = bwn0 * sum_{i,j<=512} x1b[b,x,i] W_bil[o,i,j] x2b[b,y,j]
                 + bwn1 * (x1@W_lin[:512] [b,x,o] + x2@W_lin[512:] [b,y,o] + b_lin[o])
    with x1b/x2b = x append-ones, bwn = softmax(bw).

Decomposition used here (exact):
    out[b,x,y,o] = sum_{j<512} x2[b,y,j] * UT[b,o][j,x]      (step B, PE)
                 + D1[b,o,x]                                  (rank-1 bcast + DVE add)
                 + D2[b,y,o]                                  (per-partition scalar add)
    UT[b,o][j,x] = sum_{i<512} (bwn0*W_bil[o,i,j]) * x1[b,x,i]          (step A, PE)
    D1[b,o,x]    = sum_i x1[b,x,i]*G[i,o] + g0[o],  G = bwn0*W_bil[o,:,512] + bwn1*W_lin[:512,o]
    D2[b,y,o]    = sum_j x2[b,y,j]*V[j,o],          V = bwn0*W_bil[o,512,:] + bwn1*W_lin[512:,o]
    g0[o]        = bwn0*W_bil[o,512,512] + bwn1*b_lin[o]

Sharding: tensor-parallel over O (128 output channels -> 16 per core).
Matmuls run as float32r (fp32 storage, TF32-like PE datapath, ~1e-4 rel err).

Pipelining: the PE p-state ramps 1.2->2.4 GHz only under continuous
execution, so the emission order is software-pipelined as
A(0), [A(o2+1), B(o2)] for o2 in 0..7 -- step B of iteration o2 runs on the
PE right after step A of o2+1, hiding the PSUM->SBUF eviction latency of UT
and the W-tile DMA (prefetched 3 deep on the Activation-engine queue).
The D1 term is broadcast once per (b, o2-pair) with a K=1 matmul into PSUM
and folded together with the per-partition D2 scalar in a single fused
scalar_tensor_tensor eviction.
"""

import numpy as np

import concourse.bass as bass
import concourse.mybir as mybir
import concourse.tile as tile
from concourse.bass_utils import run_bass_kernel_spmd

B, L, D, O = 4, 256, 512, 128
N_CORES = 8
O_LOC = O // N_CORES          # 16 output channels per core
N_O2 = O_LOC // 2             # 8 o-pairs per core
F32 = mybir.dt.float32
F32R = mybir.dt.float32r
ADD = mybir.AluOpType.add


# --------------------------------------------------------------------------
# Workaround: this container's walrus build accepts only ONE sync wait per
# instruction ("Too many sync wait commands").  Tile's wait assignment can
# attach several.  Post-pass: hoist extra waits onto InstEventSemaphore
# wait-carriers inserted immediately before the instruction on the same
# engine stream (same stall point, identical semantics).
_WS_CTR = [0]


def _split_multi_waits(nc):
    for f in nc.m.functions:
        for blk in f.blocks:
            insts = blk.instructions
            new = []
            changed = False
            for inst in insts:
                si = inst.sync_info
                waits = list(si.on_wait) if (si and si.on_wait) else []
                if len(waits) > 1:
                    for w in waits[:-1]:
                        _WS_CTR[0] += 1
                        carrier = mybir.InstEventSemaphore(
                            name=f"waitsplit_{_WS_CTR[0]}", ins=[], outs=[]
                        )
                        carrier.engine = inst.engine
                        carrier.sync_info = mybir.SyncInfo(on_wait=[w], on_update=[])
                        new.append(carrier)
                    si.on_wait = [waits[-1]]
                    changed = True
                new.append(inst)
            if changed:
                blk.instructions = new


# --------------------------------------------------------------------------
def build_nc(split_waits=True, n_o2=N_O2):
    nc = bass.Bass("TRN2", target_bir_lowering=False, debug=False,
                   num_devices=N_CORES)

    WM = nc.dram_tensor("WM", [O_LOC, D, D], F32R, kind="ExternalInput").ap()
    X1T = nc.dram_tensor("X1T", [D, B * L], F32R, kind="ExternalInput").ap()
    X2T = nc.dram_tensor("X2T", [D, B * L], F32R, kind="ExternalInput").ap()
    G = nc.dram_tensor("G", [D, O_LOC], F32R, kind="ExternalInput").ap()
    V = nc.dram_tensor("V", [D, O_LOC], F32R, kind="ExternalInput").ap()
    G0 = nc.dram_tensor("G0", [O_LOC, 1], F32, kind="ExternalInput").ap()
    ONES = nc.dram_tensor("ONES", [128, 128], F32R, kind="ExternalInput").ap()
    OUT = nc.dram_tensor("OUT", [B, O_LOC, L, L], F32, kind="ExternalOutput").ap()

    with tile.TileContext(nc) as tc:
        with (
            tc.tile_pool(name="const", bufs=1) as cst,
            tc.tile_pool(name="w", bufs=3) as wpool,
            tc.tile_pool(name="ut", bufs=2) as utpool,
            tc.tile_pool(name="d1bc", bufs=2) as d1pool,
            tc.tile_pool(name="cs", bufs=4) as cspool,
        ):
            # ---- resident inputs -------------------------------------------------
            X1s = cst.tile([128, 4, B * L], F32R, tag="x1s")     # [i%128, it, b*256+x]
            nc.sync.dma_start(out=X1s[:], in_=X1T.rearrange("(it p) c -> p it c", p=128))
            Gs = cst.tile([128, 4, O_LOC], F32R, tag="gs")
            nc.sync.dma_start(out=Gs[:], in_=G.rearrange("(it p) o -> p it o", p=128))
            X2s = cst.tile([128, 4, B * L], F32R, tag="x2s")     # [j%128, jt, b*256+y]
            nc.sync.dma_start(out=X2s[:], in_=X2T.rearrange("(jt p) c -> p jt c", p=128))
            Vs = cst.tile([128, 4, O_LOC], F32R, tag="vs")
            nc.sync.dma_start(out=Vs[:], in_=V.rearrange("(jt p) o -> p jt o", p=128))
            g0s = cst.tile([O_LOC, 1], F32, tag="g0s")
            nc.sync.dma_start(out=g0s[:], in_=G0[:])
            onesAll = cst.tile([128, 128], F32R, tag="ones")
            nc.sync.dma_start(out=onesAll[:], in_=ONES[:])

            # persistent D-term tiles
            D1T = [cst.tile([O_LOC, L], F32R, tag=f"d1t{b}", name=f"d1t{b}") for b in range(B)]
            D2C = [cst.tile([128, 2, O_LOC], F32, tag=f"d2c{b}", name=f"d2c{b}") for b in range(B)]
            # rhs rows for the D1 broadcast: partition 32*b holds D1T[b]
            # flattened o-major, so D1F[32b, o2*512:(o2+1)*512] covers an o-pair.
            D1F = cst.tile([128, O_LOC * L], F32R, tag="d1f")

            # W prefetch: triggered from the Activation-engine queue so the
            # trigger is not serialized behind the OUT-DMA waits on sync.
            def emit_W(o2):
                Ws = wpool.tile([128, 2, 4, D], F32R, tag="ws")   # [i%128, oi, it, j]
                nc.scalar.dma_start(
                    out=Ws[:],
                    in_=WM[2 * o2:2 * o2 + 2].rearrange("oi (it p) j -> p oi it j", p=128),
                )
                return Ws

            # ---- precompute D-terms ---------------------------------------------
            with tc.tile_pool(name="psD", bufs=1, space="PSUM") as psD:
                for b in range(B):
                    pd1 = psD.tile([O_LOC, L], F32, tag="pd1")
                    for it in range(4):
                        nc.tensor.matmul(
                            pd1[:],
                            lhsT=Gs[:, it, :],
                            rhs=X1s[:, it, b * L:(b + 1) * L],
                            start=(it == 0), stop=(it == 3),
                        )
                    nc.vector.tensor_scalar_add(D1T[b][:], pd1[:], g0s[:, 0:1])
                    for o in range(O_LOC):
                        nc.sync.dma_start(
                            out=D1F[32 * b:32 * b + 1, o * L:(o + 1) * L],
                            in_=D1T[b][o:o + 1, :],
                        )
                    for yt in range(2):
                        pd2 = psD.tile([128, O_LOC], F32, tag="pd2")
                        for jt in range(4):
                            nc.tensor.matmul(
                                pd2[:],
                                lhsT=X2s[:, jt, b * L + yt * 128: b * L + (yt + 1) * 128],
                                rhs=Vs[:, jt, :],
                                start=(jt == 0), stop=(jt == 3),
                            )
                        nc.vector.tensor_copy(out=D2C[b][:, yt, :], in_=pd2[:])

            # ---- main loop over o-pairs, software-pipelined ----------------------
            ctx_psA = tc.tile_pool(name="psA", bufs=3, space="PSUM")
            ctx_psB = tc.tile_pool(name="psB", bufs=3, space="PSUM")
            ctx_psb = tc.tile_pool(name="psbc", bufs=2, space="PSUM")
            psA = ctx_psA.__enter__()
            psB = ctx_psB.__enter__()
            psBC = ctx_psb.__enter__()

            def emit_A(o2, Ws):
                # step A: UT[bp][j, (jt, b2, oi, x)] for this o-pair; the
                # b2-major free layout keeps step B's rhs slice contiguous.
                UT = [utpool.tile([128, 4, 2, 2, 256], F32R, tag=f"utp{bp}", name=f"utp{bp}")
                      for bp in range(2)]
                for oi in range(2):
                    for jt in range(4):
                        for bp in range(2):
                            pa = psA.tile([128, 512], F32, tag="pa")
                            for it in range(4):
                                nc.tensor.matmul(
                                    pa[:],
                                    lhsT=Ws[:, oi, it, jt * 128:(jt + 1) * 128],
                                    rhs=X1s[:, it, bp * 512:(bp + 1) * 512],
                                    start=(it == 0), stop=(it == 3),
                                )
                            nc.vector.tensor_copy(
                                out=UT[bp][:, jt, :, oi, :], in_=pa[:])
                return UT

            def emit_B(o2, UT):
                # step B: out[y, (oi, x)] per (b, yt)
                for b in range(B):
                    # D1 broadcast for this (b, o-pair): rank-1 ones x D1row,
                    # then PSUM->SBUF eviction on the (otherwise idle) ACT
                    # engine so the fused STT below reads only one PSUM input.
                    pbc = psBC.tile([128, 512], F32, tag="pbc")
                    nc.tensor.matmul(
                        pbc[:],
                        lhsT=onesAll[32 * b:32 * b + 1, 0:128],
                        rhs=D1F[32 * b:32 * b + 1, o2 * 512:(o2 + 1) * 512],
                        start=True, stop=True,
                        tile_position=(32 * b, 0),
                    )
                    d1bc = d1pool.tile([128, 512], F32, tag="d1bc")
                    nc.scalar.activation(
                        out=d1bc[:], in_=pbc[:],
                        func=mybir.ActivationFunctionType.Copy,
                    )
                    bp, b2 = divmod(b, 2)
                    for yt in range(2):
                        pb = psB.tile([128, 512], F32, tag="pb")
                        for jt in range(4):
                            nc.tensor.matmul(
                                pb[:],
                                lhsT=X2s[:, jt, b * L + yt * 128: b * L + (yt + 1) * 128],
                                rhs=UT[bp][:, jt, b2, :, :],
                                start=(jt == 0), stop=(jt == 3),
                            )
                        cs = cspool.tile([128, 512], F32, tag="cs")
                        for oi in range(2):
                            og = 2 * o2 + oi
                            # cs = (pb + D2[y,o]) + D1bc[o,x], single DVE op
                            nc.vector.scalar_tensor_tensor(
                                out=cs[:, oi * 256:(oi + 1) * 256],
                                in0=pb[:, oi * 256:(oi + 1) * 256],
                                scalar=D2C[b][:, yt, og:og + 1],
                                in1=d1bc[:, oi * 256:(oi + 1) * 256],
                                op0=ADD, op1=ADD,
                            )
                            nc.sync.dma_start(
                                out=OUT[b, og, yt * 128:(yt + 1) * 128, :],
                                in_=cs[:, oi * 256:(oi + 1) * 256],
                            )

            WsQ = [emit_W(i) for i in range(min(3, n_o2))]
            UT_prev = emit_A(0, WsQ[0])
            for o2 in range(n_o2):
                if o2 + 3 < n_o2:
                    WsQ.append(emit_W(o2 + 3))
                UT_next = emit_A(o2 + 1, WsQ[o2 + 1]) if o2 + 1 < n_o2 else None
                emit_B(o2, UT_prev)
                UT_prev = UT_next

            ctx_psb.__exit__(None, None, None)
            ctx_psB.__exit__(None, None, None)
            ctx_psA.__exit__(None, None, None)

    if split_waits:
        _split_multi_waits(nc)
    return nc


_NC_CACHE = None


def _get_nc():
    global _NC_CACHE
    if _NC_CACHE is None:
        _NC_CACHE = build_nc()
    return _NC_CACHE


def _prep_inputs(x1, x2, bw, W_bil, W_lin, b_lin):
    """Host-side glue: softmax of the 2-vector, per-core slicing/layout."""
    x1 = np.asarray(x1, np.float32)
    x2 = np.asarray(x2, np.float32)
    bw = np.asarray(bw, np.float64)
    W_bil = np.asarray(W_bil, np.float32)
    W_lin = np.asarray(W_lin, np.float32)
    b_lin = np.asarray(b_lin, np.float32)

    e = np.exp(bw - bw.max())
    bwn = (e / e.sum()).astype(np.float32)
    bwn0, bwn1 = float(bwn[0]), float(bwn[1])

    x1T = np.ascontiguousarray(x1.transpose(2, 0, 1).reshape(D, B * L))
    x2T = np.ascontiguousarray(x2.transpose(2, 0, 1).reshape(D, B * L))
    ones = np.ones((128, 128), np.float32)

    in_maps = []
    for c in range(N_CORES):
        o_sl = slice(c * O_LOC, (c + 1) * O_LOC)
        Wb = W_bil[o_sl]                                   # [16, 513, 513]
        WM = np.ascontiguousarray(bwn0 * Wb[:, :D, :D])
        G = np.ascontiguousarray(bwn0 * Wb[:, :D, D].T + bwn1 * W_lin[:D, o_sl])
        V = np.ascontiguousarray(bwn0 * Wb[:, D, :D].T + bwn1 * W_lin[D:, o_sl])
        G0 = (bwn0 * Wb[:, D, D] + bwn1 * b_lin[o_sl]).reshape(O_LOC, 1)
        in_maps.append({
            "WM": WM, "X1T": x1T, "X2T": x2T,
            "G": G.astype(np.float32), "V": V.astype(np.float32),
            "G0": np.ascontiguousarray(G0, dtype=np.float32), "ONES": ones,
        })
    return in_maps


def _assemble(results):
    out = np.empty((B, L, L, O), np.float32)
    for c in range(N_CORES):
        # per-core OUT is [b, o_local, y, x] -> full is [b, x, y, o]
        out[:, :, :, c * O_LOC:(c + 1) * O_LOC] = \
            results[c]["OUT"].transpose(0, 3, 2, 1)
    return out


def kernel(**inputs):
    in_maps = _prep_inputs(**inputs)
    nc = _get_nc()
    res = run_bass_kernel_spmd(nc, in_maps, list(range(N_CORES)))
    return _assemble(res.results)


# revision 8
# speedup vs baseline: 1.1296x; 1.0416x over previous
"""Biaffine kernel for Trainium2, 8-core SPMD.

Math (reference):
    out[b,x,y,o] = bwn0 * sum_{i,j<=512} x1b[b,x,i] W_bil[o,i,j] x2b[b,y,j]
                 + bwn1 * (x1@W_lin[:512] [b,x,o] + x2@W_lin[512:] [b,y,o] + b_lin[o])
    with x1b/x2b = x append-ones, bwn = softmax(bw).

Decomposition (exact):
    out[b,x,y,o] = sum_{j<512} x2[b,y,j] * UT[b,o][j,x]      (step B, PE)
                 + D1[b,o,x] + D2[b,y,o]                      (fused DVE eviction)
    UT[b,o][j,x] = sum_{i<512} (bwn0*W_bil[o,i,j]) * x1[b,x,i]   (step A, PE)
    D1, D2 are cheap rank-2 terms (x1@G+g0, x2@V) computed on the HOST and
    shipped as inputs: D1 pre-replicated across the 128 partitions so the
    y-partitioned step-B eviction can read it directly, D2 in per-partition
    scalar layout.  This keeps the PE stream pure 512-row matmuls.

Sharding: tensor-parallel over O (128 output channels -> 16 per core).
Matmuls run as float32r (fp32 storage, TF32-like PE datapath, ~1e-4 rel err).

Pipelining: the PE p-state ramps 1.2->2.4 GHz only under continuous
execution, so emission is software-pipelined as A(0), [A(o2+1), B(o2)] --
step B of o2 runs right after step A of o2+1, hiding UT eviction latency.
W tiles prefetch 3 deep on the ACT-engine queue, D1 slabs 2 deep on the
GpSimd queue, so no trigger serializes behind the OUT-DMA waits on sync.
"""

import numpy as np

import concourse.bass as bass
import concourse.mybir as mybir
import concourse.tile as tile
from concourse.bass_utils import run_bass_kernel_spmd

B, L, D, O = 4, 256, 512, 128
N_CORES = 8
O_LOC = O // N_CORES          # 16 output channels per core
N_O2 = O_LOC // 2             # 8 o-pairs per core
F32 = mybir.dt.float32
F32R = mybir.dt.float32r
ADD = mybir.AluOpType.add


# --------------------------------------------------------------------------
# Workaround: this container's walrus build accepts only ONE sync wait per
# instruction ("Too many sync wait commands").  Tile's wait assignment can
# attach several.  Post-pass: hoist extra waits onto InstEventSemaphore
# wait-carriers inserted immediately before the instruction on the same
# engine stream (same stall point, identical semantics).
_WS_CTR = [0]


def _split_multi_waits(nc):
    for f in nc.m.functions:
        for blk in f.blocks:
            insts = blk.instructions
            new = []
            changed = False
            for inst in insts:
                si = inst.sync_info
                waits = list(si.on_wait) if (si and si.on_wait) else []
                if len(waits) > 1:
                    for w in waits[:-1]:
                        _WS_CTR[0] += 1
                        carrier = mybir.InstEventSemaphore(
                            name=f"waitsplit_{_WS_CTR[0]}", ins=[], outs=[]
                        )
                        carrier.engine = inst.engine
                        carrier.sync_info = mybir.SyncInfo(on_wait=[w], on_update=[])
                        new.append(carrier)
                    si.on_wait = [waits[-1]]
                    changed = True
                new.append(inst)
            if changed:
                blk.instructions = new


# --------------------------------------------------------------------------
def build_nc(split_waits=True, n_o2=N_O2):
    nc = bass.Bass("TRN2", target_bir_lowering=False, debug=False,
                   num_devices=N_CORES)

    WM = nc.dram_tensor("WM", [O_LOC, D, D], F32R, kind="ExternalInput").ap()
    X1T = nc.dram_tensor("X1T", [D, B * L], F32R, kind="ExternalInput").ap()
    X2T = nc.dram_tensor("X2T", [D, B * L], F32R, kind="ExternalInput").ap()
    # D1 replicated across partitions: [o2, p, (b, oi*256+x)]
    D1B = nc.dram_tensor("D1B", [N_O2, 128, B * 512], F32, kind="ExternalInput").ap()
    # D2 in per-partition scalar layout: [b, p=y%128, (yt, o)]
    D2H = nc.dram_tensor("D2H", [B, 128, 2 * O_LOC], F32, kind="ExternalInput").ap()
    OUT = nc.dram_tensor("OUT", [B, O_LOC, L, L], F32, kind="ExternalOutput").ap()

    with tile.TileContext(nc) as tc:
        with (
            tc.tile_pool(name="const", bufs=1) as cst,
            tc.tile_pool(name="w", bufs=3) as wpool,
            tc.tile_pool(name="ut", bufs=2) as utpool,
            tc.tile_pool(name="d1", bufs=2) as d1pool,
            tc.tile_pool(name="cs", bufs=4) as cspool,
        ):
            # ---- resident inputs --------------------------------------------
            X1s = cst.tile([128, 4, B * L], F32R, tag="x1s")     # [i%128, it, b*256+x]
            nc.sync.dma_start(out=X1s[:], in_=X1T.rearrange("(it p) c -> p it c", p=128))
            X2s = cst.tile([128, 4, B * L], F32R, tag="x2s")     # [j%128, jt, b*256+y]
            nc.sync.dma_start(out=X2s[:], in_=X2T.rearrange("(jt p) c -> p jt c", p=128))
            D2C = [cst.tile([128, 2, O_LOC], F32, tag=f"d2c{b}", name=f"d2c{b}")
                   for b in range(B)]
            for b in range(B):
                nc.sync.dma_start(
                    out=D2C[b][:], in_=D2H[b].rearrange("p (yt o) -> p yt o", yt=2))

            def emit_W(o2):
                Ws = wpool.tile([128, 2, 4, D], F32R, tag="ws")   # [i%128, oi, it, j]
                nc.scalar.dma_start(
                    out=Ws[:],
                    in_=WM[2 * o2:2 * o2 + 2].rearrange("oi (it p) j -> p oi it j", p=128),
                )
                return Ws

            def emit_D1(o2):
                d1t = d1pool.tile([128, B * 512], F32, tag="d1t")
                nc.gpsimd.dma_start(out=d1t[:], in_=D1B[o2])
                return d1t

            # ---- main loop over o-pairs, software-pipelined ------------------
            ctx_psA = tc.tile_pool(name="psA", bufs=4, space="PSUM")
            ctx_psB = tc.tile_pool(name="psB", bufs=4, space="PSUM")
            psA = ctx_psA.__enter__()
            psB = ctx_psB.__enter__()

            def emit_A(o2, Ws):
                # step A: UT[bp][j, (jt, b2, oi, x)] for this o-pair; the
                # b2-major free layout keeps step B's rhs slice contiguous.
                UT = [utpool.tile([128, 4, 2, 2, 256], F32R, tag=f"utp{bp}", name=f"utp{bp}")
                      for bp in range(2)]
                for oi in range(2):
                    for jt in range(4):
                        for bp in range(2):
                            pa = psA.tile([128, 512], F32, tag="pa")
                            for it in range(4):
                                nc.tensor.matmul(
                                    pa[:],
                                    lhsT=Ws[:, oi, it, jt * 128:(jt + 1) * 128],
                                    rhs=X1s[:, it, bp * 512:(bp + 1) * 512],
                                    start=(it == 0), stop=(it == 3),
                                )
                            nc.vector.tensor_copy(
                                out=UT[bp][:, jt, :, oi, :], in_=pa[:])
                return UT

            def emit_B(o2, UT, d1t):
                # step B: out[y, (oi, x)] per (b, yt)
                for b in range(B):
                    bp, b2 = divmod(b, 2)
                    for yt in range(2):
                        pb = psB.tile([128, 512], F32, tag="pb")
                        for jt in range(4):
                            nc.tensor.matmul(
                                pb[:],
                                lhsT=X2s[:, jt, b * L + yt * 128: b * L + (yt + 1) * 128],
                                rhs=UT[bp][:, jt, b2, :, :],
                                start=(jt == 0), stop=(jt == 3),
                            )
                        cs = cspool.tile([128, 512], F32, tag="cs")
                        for oi in range(2):
                            og = 2 * o2 + oi
                            # cs = (pb + D2[y,o]) + D1[o,x], single DVE op
                            nc.vector.scalar_tensor_tensor(
                                out=cs[:, oi * 256:(oi + 1) * 256],
                                in0=pb[:, oi * 256:(oi + 1) * 256],
                                scalar=D2C[b][:, yt, og:og + 1],
                                in1=d1t[:, b * 512 + oi * 256: b * 512 + (oi + 1) * 256],
                                op0=ADD, op1=ADD,
                            )
                            nc.sync.dma_start(
                                out=OUT[b, og, yt * 128:(yt + 1) * 128, :],
                                in_=cs[:, oi * 256:(oi + 1) * 256],
                            )

            WsQ = [emit_W(i) for i in range(min(3, n_o2))]
            D1Q = [emit_D1(i) for i in range(min(2, n_o2))]
            UT_prev = emit_A(0, WsQ[0])
            for o2 in range(n_o2):
                if o2 + 3 < n_o2:
                    WsQ.append(emit_W(o2 + 3))
                if o2 + 2 < n_o2:
                    D1Q.append(emit_D1(o2 + 2))
                UT_next = emit_A(o2 + 1, WsQ[o2 + 1]) if o2 + 1 < n_o2 else None
                emit_B(o2, UT_prev, D1Q[o2])
                UT_prev = UT_next

            ctx_psB.__exit__(None, None, None)
            ctx_psA.__exit__(None, None, None)

    if split_waits:
        _split_multi_waits(nc)
    return nc


_NC_CACHE = None


def _get_nc():
    global _NC_CACHE
    if _NC_CACHE is None:
        _NC_CACHE = build_nc()
    return _NC_CACHE


def _prep_inputs(x1, x2, bw, W_bil, W_lin, b_lin):
    """Host-side glue: softmax of the 2-vector, per-core slicing/layout,
    and the cheap rank-2 D-terms (D1 = x1@G+g0, D2 = x2@V)."""
    x1 = np.asarray(x1, np.float32)
    x2 = np.asarray(x2, np.float32)
    bw = np.asarray(bw, np.float64)
    W_bil = np.asarray(W_bil, np.float32)
    W_lin = np.asarray(W_lin, np.float32)
    b_lin = np.asarray(b_lin, np.float32)

    e = np.exp(bw - bw.max())
    bwn = (e / e.sum()).astype(np.float32)
    bwn0, bwn1 = float(bwn[0]), float(bwn[1])

    x1T = np.ascontiguousarray(x1.transpose(2, 0, 1).reshape(D, B * L))
    x2T = np.ascontiguousarray(x2.transpose(2, 0, 1).reshape(D, B * L))

    in_maps = []
    for c in range(N_CORES):
        o_sl = slice(c * O_LOC, (c + 1) * O_LOC)
        Wb = W_bil[o_sl]                                   # [16, 513, 513]
        WM = np.ascontiguousarray(bwn0 * Wb[:, :D, :D])
        G = bwn0 * Wb[:, :D, D].T + bwn1 * W_lin[:D, o_sl]     # [D, 16]
        V = bwn0 * Wb[:, D, :D].T + bwn1 * W_lin[D:, o_sl]     # [D, 16]
        g0 = bwn0 * Wb[:, D, D] + bwn1 * b_lin[o_sl]           # [16]

        D1 = np.einsum('bxd,do->bxo', x1, G) + g0              # [B, L, 16]
        D2 = np.einsum('byd,do->byo', x2, V)                   # [B, L, 16]
        # D1B[o2, p, (b, oi*256+x)] = D1[b, x, 2*o2+oi], replicated over p
        arr = D1.transpose(2, 0, 1).reshape(N_O2, 2, B, L)     # [o2, oi, b, x]
        arr = arr.transpose(0, 2, 1, 3).reshape(N_O2, 1, B * 512)
        D1Bv = np.ascontiguousarray(
            np.broadcast_to(arr, (N_O2, 128, B * 512)), dtype=np.float32)
        # D2H[b, p, (yt, o)] = D2[b, yt*128+p, o]
        D2Hv = np.ascontiguousarray(
            D2.reshape(B, 2, 128, O_LOC).transpose(0, 2, 1, 3)
            .reshape(B, 128, 2 * O_LOC), dtype=np.float32)

        in_maps.append({
            "WM": WM, "X1T": x1T, "X2T": x2T,
            "D1B": D1Bv, "D2H": D2Hv,
        })
    return in_maps


def _assemble(results):
    out = np.empty((B, L, L, O), np.float32)
    for c in range(N_CORES):
        # per-core OUT is [b, o_local, y, x] -> full is [b, x, y, o]
        out[:, :, :, c * O_LOC:(c + 1) * O_LOC] = \
            results[c]["OUT"].transpose(0, 3, 2, 1)
    return out


def kernel(**inputs):
    in_maps = _prep_inputs(**inputs)
    nc = _get_nc()
    res = run_bass_kernel_spmd(nc, in_maps, list(range(N_CORES)))
    return _assemble(res.results)


# revision 16
# speedup vs baseline: 1.3247x; 1.1727x over previous
"""Biaffine kernel for Trainium2, 8-core SPMD.

Math (reference):
    out[b,x,y,o] = bwn0 * sum_{i,j<=512} x1b[b,x,i] W_bil[o,i,j] x2b[b,y,j]
                 + bwn1 * (x1@W_lin[:512] [b,x,o] + x2@W_lin[512:] [b,y,o] + b_lin[o])
    with x1b/x2b = x append-ones, bwn = softmax(bw).

Decomposition (exact):
    out[b,x,y,o] = sum_{j<512} x2[b,y,j] * UT[b,o][j,x]      (step B, PE)
                 + D1[b,o,x] + D2[b,y,o]                      (fused DVE eviction)
    UT[b,o][j,x] = sum_{i<512} (bwn0*W_bil[o,i,j]) * x1[b,x,i]   (step A, PE)
    D1, D2 are cheap rank-2 terms (x1@G+g0, x2@V) computed on the HOST and
    shipped as inputs: D1 pre-replicated across the 128 partitions so the
    y-partitioned step-B eviction can read it directly, D2 in per-partition
    scalar layout.  This keeps the PE stream pure 512-row matmuls.

Sharding: tensor-parallel over O (128 output channels -> 16 per core).
Matmuls run as float32r (fp32 storage, TF32-like PE datapath, ~1e-4 rel err).

Pipelining: the PE p-state ramps 1.2->2.4 GHz only under continuous
execution, so emission is software-pipelined as A(0), [A(o2+1), B(o2)] --
step B of o2 runs right after step A of o2+1, hiding UT eviction latency.
W tiles prefetch 3 deep on the ACT-engine queue, D1 slabs 2 deep on the
GpSimd queue, so no trigger serializes behind the OUT-DMA waits on sync.
"""

import ml_dtypes
import numpy as np

import concourse.bass as bass
import concourse.mybir as mybir
import concourse.tile as tile
from concourse.bass_utils import run_bass_kernel_spmd

B, L, D, O = 4, 256, 512, 128
N_CORES = 8
O_LOC = O // N_CORES          # 16 output channels per core
N_O2 = O_LOC // 2             # 8 o-pairs per core
F32 = mybir.dt.float32
F32R = mybir.dt.float32r
BF16 = mybir.dt.bfloat16
ADD = mybir.AluOpType.add


# --------------------------------------------------------------------------
# Workaround: this container's walrus build accepts only ONE sync wait per
# instruction ("Too many sync wait commands").  Tile's wait assignment can
# attach several.  Post-pass: hoist extra waits onto InstEventSemaphore
# wait-carriers inserted immediately before the instruction on the same
# engine stream (same stall point, identical semantics).
_WS_CTR = [0]


def _split_multi_waits(nc):
    for f in nc.m.functions:
        for blk in f.blocks:
            insts = blk.instructions
            new = []
            changed = False
            for inst in insts:
                si = inst.sync_info
                waits = list(si.on_wait) if (si and si.on_wait) else []
                if len(waits) > 1:
                    for w in waits[:-1]:
                        _WS_CTR[0] += 1
                        carrier = mybir.InstEventSemaphore(
                            name=f"waitsplit_{_WS_CTR[0]}", ins=[], outs=[]
                        )
                        carrier.engine = inst.engine
                        carrier.sync_info = mybir.SyncInfo(on_wait=[w], on_update=[])
                        new.append(carrier)
                    si.on_wait = [waits[-1]]
                    changed = True
                new.append(inst)
            if changed:
                blk.instructions = new


# --------------------------------------------------------------------------
def build_nc(split_waits=True, n_o2=N_O2):
    nc = bass.Bass("TRN2", target_bir_lowering=False, debug=False,
                   num_devices=N_CORES)

    WM = nc.dram_tensor("WM", [O_LOC, D, D], F32R, kind="ExternalInput").ap()
    X1T = nc.dram_tensor("X1T", [D, B * L], F32R, kind="ExternalInput").ap()
    X2T = nc.dram_tensor("X2T", [D, B * L], BF16, kind="ExternalInput").ap()
    # D1 replicated across partitions: [o2, p, (b, oi*256+x)]
    D1B = nc.dram_tensor("D1B", [N_O2, 128, B * 512], F32, kind="ExternalInput").ap()
    # D2 in per-partition scalar layout: [b, p=y%128, (yt, o)]
    D2H = nc.dram_tensor("D2H", [B, 128, 2 * O_LOC], F32, kind="ExternalInput").ap()
    OUT = nc.dram_tensor("OUT", [B, O_LOC, L, L], F32, kind="ExternalOutput").ap()

    with tile.TileContext(nc) as tc:
        with (
            tc.tile_pool(name="const", bufs=1) as cst,
            tc.tile_pool(name="w", bufs=3) as wpool,
            tc.tile_pool(name="ut", bufs=2) as utpool,
            tc.tile_pool(name="d1", bufs=2) as d1pool,
            tc.tile_pool(name="cs", bufs=8) as cspool,
        ):
            # ---- resident inputs --------------------------------------------
            # A(0) only needs X1 (sync ring) + W0 (scalar ring); everything
            # else is emitted after A(0) so it doesn't steal DMA bandwidth
            # from the critical startup path.
            X1s = cst.tile([128, 4, B * L], F32R, tag="x1s")     # [i%128, it, b*256+x]
            nc.sync.dma_start(out=X1s[:], in_=X1T.rearrange("(it p) c -> p it c", p=128))
            X2s = cst.tile([128, 4, B * L], BF16, tag="x2s")     # [j%128, jt, b*256+y]
            D2C = [cst.tile([128, 2, O_LOC], F32, tag=f"d2c{b}", name=f"d2c{b}")
                   for b in range(B)]

            def emit_late_loads():
                nc.sync.dma_start(out=X2s[:], in_=X2T.rearrange("(jt p) c -> p jt c", p=128))
                for b in range(B):
                    nc.sync.dma_start(
                        out=D2C[b][:], in_=D2H[b].rearrange("p (yt o) -> p yt o", yt=2))

            def emit_W(o2):
                Ws = wpool.tile([128, 2, 4, D], F32R, tag="ws")   # [i%128, oi, it, j]
                nc.scalar.dma_start(
                    out=Ws[:],
                    in_=WM[2 * o2:2 * o2 + 2].rearrange("oi (it p) j -> p oi it j", p=128),
                )
                return Ws

            def emit_D1(o2):
                d1t = d1pool.tile([128, B * 512], F32, tag="d1t")
                nc.gpsimd.dma_start(out=d1t[:], in_=D1B[o2])
                return d1t

            # ---- main loop over o-pairs, software-pipelined ------------------
            ctx_psA = tc.tile_pool(name="psA", bufs=4, space="PSUM")
            ctx_psB = tc.tile_pool(name="psB", bufs=4, space="PSUM")
            psA = ctx_psA.__enter__()
            psB = ctx_psB.__enter__()

            def emit_A(o2, Ws):
                # step A: UT[bp][j, (jt, b2, oi, x)] for this o-pair; the
                # b2-major free layout keeps step B's rhs slice contiguous.
                UT = [utpool.tile([128, 4, 2, 2, 256], BF16, tag=f"utp{bp}", name=f"utp{bp}")
                      for bp in range(2)]
                for oi in range(2):
                    for jt in range(4):
                        for bp in range(2):
                            pa = psA.tile([128, 512], F32, tag="pa")
                            for it in range(4):
                                nc.tensor.matmul(
                                    pa[:],
                                    lhsT=Ws[:, oi, it, jt * 128:(jt + 1) * 128],
                                    rhs=X1s[:, it, bp * 512:(bp + 1) * 512],
                                    start=(it == 0), stop=(it == 3),
                                )
                            nc.vector.tensor_copy(
                                out=UT[bp][:, jt, :, oi, :], in_=pa[:])
                return UT

            def emit_B(o2, UT, d1t):
                # step B: out[y, (oi, x)] per (b, yt)
                for b in range(B):
                    bp, b2 = divmod(b, 2)
                    for yt in range(2):
                        pb = psB.tile([128, 512], F32, tag="pb")
                        for jt in range(4):
                            nc.tensor.matmul(
                                pb[:],
                                lhsT=X2s[:, jt, b * L + yt * 128: b * L + (yt + 1) * 128],
                                rhs=UT[bp][:, jt, b2, :, :],
                                start=(jt == 0), stop=(jt == 3),
                            )
                        cs = cspool.tile([128, 512], F32, tag="cs")
                        for oi in range(2):
                            og = 2 * o2 + oi
                            # cs = (pb + D2[y,o]) + D1[o,x], single DVE op
                            nc.vector.scalar_tensor_tensor(
                                out=cs[:, oi * 256:(oi + 1) * 256],
                                in0=pb[:, oi * 256:(oi + 1) * 256],
                                scalar=D2C[b][:, yt, og:og + 1],
                                in1=d1t[:, b * 512 + oi * 256: b * 512 + (oi + 1) * 256],
                                op0=ADD, op1=ADD,
                            )
                        nc.sync.dma_start(
                            out=OUT[b, 2 * o2:2 * o2 + 2, yt * 128:(yt + 1) * 128, :]
                                .rearrange("og y x -> y og x"),
                            in_=cs[:],
                        )

            WsQ = [emit_W(i) for i in range(min(3, n_o2))]
            UT_prev = emit_A(0, WsQ[0])
            emit_late_loads()
            D1Q = [emit_D1(i) for i in range(min(2, n_o2))]
            for o2 in range(n_o2):
                if o2 + 3 < n_o2:
                    WsQ.append(emit_W(o2 + 3))
                if o2 + 2 < n_o2:
                    D1Q.append(emit_D1(o2 + 2))
                UT_next = emit_A(o2 + 1, WsQ[o2 + 1]) if o2 + 1 < n_o2 else None
                emit_B(o2, UT_prev, D1Q[o2])
                UT_prev = UT_next

            ctx_psB.__exit__(None, None, None)
            ctx_psA.__exit__(None, None, None)

    if split_waits:
        _split_multi_waits(nc)
    return nc


_NC_CACHE = None


def _get_nc():
    global _NC_CACHE
    if _NC_CACHE is None:
        _NC_CACHE = build_nc()
    return _NC_CACHE


def _prep_inputs(x1, x2, bw, W_bil, W_lin, b_lin):
    """Host-side glue: softmax of the 2-vector, per-core slicing/layout,
    and the cheap rank-2 D-terms (D1 = x1@G+g0, D2 = x2@V)."""
    x1 = np.asarray(x1, np.float32)
    x2 = np.asarray(x2, np.float32)
    bw = np.asarray(bw, np.float64)
    W_bil = np.asarray(W_bil, np.float32)
    W_lin = np.asarray(W_lin, np.float32)
    b_lin = np.asarray(b_lin, np.float32)

    e = np.exp(bw - bw.max())
    bwn = (e / e.sum()).astype(np.float32)
    bwn0, bwn1 = float(bwn[0]), float(bwn[1])

    x1T = np.ascontiguousarray(x1.transpose(2, 0, 1).reshape(D, B * L))
    x2T = np.ascontiguousarray(
        x2.transpose(2, 0, 1).reshape(D, B * L).astype(ml_dtypes.bfloat16))

    in_maps = []
    for c in range(N_CORES):
        o_sl = slice(c * O_LOC, (c + 1) * O_LOC)
        Wb = W_bil[o_sl]                                   # [16, 513, 513]
        WM = np.ascontiguousarray(bwn0 * Wb[:, :D, :D])
        G = bwn0 * Wb[:, :D, D].T + bwn1 * W_lin[:D, o_sl]     # [D, 16]
        V = bwn0 * Wb[:, D, :D].T + bwn1 * W_lin[D:, o_sl]     # [D, 16]
        g0 = bwn0 * Wb[:, D, D] + bwn1 * b_lin[o_sl]           # [16]

        D1 = np.einsum('bxd,do->bxo', x1, G) + g0              # [B, L, 16]
        D2 = np.einsum('byd,do->byo', x2, V)                   # [B, L, 16]
        # D1B[o2, p, (b, oi*256+x)] = D1[b, x, 2*o2+oi], replicated over p
        arr = D1.transpose(2, 0, 1).reshape(N_O2, 2, B, L)     # [o2, oi, b, x]
        arr = arr.transpose(0, 2, 1, 3).reshape(N_O2, 1, B * 512)
        D1Bv = np.ascontiguousarray(
            np.broadcast_to(arr, (N_O2, 128, B * 512)), dtype=np.float32)
        # D2H[b, p, (yt, o)] = D2[b, yt*128+p, o]
        D2Hv = np.ascontiguousarray(
            D2.reshape(B, 2, 128, O_LOC).transpose(0, 2, 1, 3)
            .reshape(B, 128, 2 * O_LOC), dtype=np.float32)

        in_maps.append({
            "WM": WM, "X1T": x1T, "X2T": x2T,
            "D1B": D1Bv, "D2H": D2Hv,
        })
    return in_maps


def _assemble(results):
    out = np.empty((B, L, L, O), np.float32)
    for c in range(N_CORES):
        # per-core OUT is [b, o_local, y, x] -> full is [b, x, y, o]
        out[:, :, :, c * O_LOC:(c + 1) * O_LOC] = \
            results[c]["OUT"].transpose(0, 3, 2, 1)
    return out


def kernel(**inputs):
    in_maps = _prep_inputs(**inputs)
    nc = _get_nc()
    res = run_bass_kernel_spmd(nc, in_maps, list(range(N_CORES)))
    return _assemble(res.results)


# revision 19
# speedup vs baseline: 1.3928x; 1.0514x over previous
"""Biaffine kernel for Trainium2, 8-core SPMD.

Math (reference):
    out[b,x,y,o] = bwn0 * sum_{i,j<=512} x1b[b,x,i] W_bil[o,i,j] x2b[b,y,j]
                 + bwn1 * (x1@W_lin[:512] [b,x,o] + x2@W_lin[512:] [b,y,o] + b_lin[o])
    with x1b/x2b = x append-ones, bwn = softmax(bw).

Decomposition (exact):
    out[b,x,y,o] = sum_{j<512} x2[b,y,j] * UT[b,o][j,x]      (step B, PE)
                 + D1[b,o,x] + D2[b,y,o]                      (fused DVE eviction)
    UT[b,o][j,x] = sum_{i<512} (bwn0*W_bil[o,i,j]) * x1[b,x,i]   (step A, PE)
    D1, D2 are cheap rank-2 terms (x1@G+g0, x2@V) computed on the HOST and
    shipped as inputs: D1 pre-replicated across the 128 partitions so the
    y-partitioned step-B eviction can read it directly, D2 in per-partition
    scalar layout.  This keeps the PE stream pure 512-row matmuls.

Sharding: tensor-parallel over O (128 output channels -> 16 per core).
Matmuls run as float32r (fp32 storage, TF32-like PE datapath, ~1e-4 rel err).

Pipelining: the PE p-state ramps 1.2->2.4 GHz only under continuous
execution, so emission is software-pipelined as A(0), [A(o2+1), B(o2)] --
step B of o2 runs right after step A of o2+1, hiding UT eviction latency.
W tiles prefetch 3 deep on the ACT-engine queue, D1 slabs 2 deep on the
GpSimd queue, so no trigger serializes behind the OUT-DMA waits on sync.
"""

import ml_dtypes
import numpy as np

import concourse.bass as bass
import concourse.mybir as mybir
import concourse.tile as tile
from concourse.bass_utils import run_bass_kernel_spmd

B, L, D, O = 4, 256, 512, 128
N_CORES = 8
O_LOC = O // N_CORES          # 16 output channels per core
N_O2 = O_LOC // 2             # 8 o-pairs per core
F32 = mybir.dt.float32
F32R = mybir.dt.float32r
BF16 = mybir.dt.bfloat16
ADD = mybir.AluOpType.add


# --------------------------------------------------------------------------
# Workaround: this container's walrus build accepts only ONE sync wait per
# instruction ("Too many sync wait commands").  Tile's wait assignment can
# attach several.  Post-pass: hoist extra waits onto InstEventSemaphore
# wait-carriers inserted immediately before the instruction on the same
# engine stream (same stall point, identical semantics).
_WS_CTR = [0]


def _split_multi_waits(nc):
    for f in nc.m.functions:
        for blk in f.blocks:
            insts = blk.instructions
            new = []
            changed = False
            for inst in insts:
                si = inst.sync_info
                waits = list(si.on_wait) if (si and si.on_wait) else []
                if len(waits) > 1:
                    for w in waits[:-1]:
                        _WS_CTR[0] += 1
                        carrier = mybir.InstEventSemaphore(
                            name=f"waitsplit_{_WS_CTR[0]}", ins=[], outs=[]
                        )
                        carrier.engine = inst.engine
                        carrier.sync_info = mybir.SyncInfo(on_wait=[w], on_update=[])
                        new.append(carrier)
                    si.on_wait = [waits[-1]]
                    changed = True
                new.append(inst)
            if changed:
                blk.instructions = new


# --------------------------------------------------------------------------
def build_nc(split_waits=True, n_o2=N_O2):
    nc = bass.Bass("TRN2", target_bir_lowering=False, debug=False,
                   num_devices=N_CORES)

    WM = nc.dram_tensor("WM", [O_LOC, D, D], F32R, kind="ExternalInput").ap()
    X1T = nc.dram_tensor("X1T", [D, B * L], F32R, kind="ExternalInput").ap()
    X2T = nc.dram_tensor("X2T", [D, B * L], BF16, kind="ExternalInput").ap()
    # D1 replicated across partitions: [o2, p, (b, oi*256+x)]
    D1B = nc.dram_tensor("D1B", [N_O2, 128, B * 512], F32, kind="ExternalInput").ap()
    # D2 in per-partition scalar layout: [b, p=y%128, (yt, o)]
    D2H = nc.dram_tensor("D2H", [B, 128, 2 * O_LOC], F32, kind="ExternalInput").ap()
    OUT = nc.dram_tensor("OUT", [B, O_LOC, L, L], F32, kind="ExternalOutput").ap()

    with tile.TileContext(nc) as tc:
        with (
            tc.tile_pool(name="const", bufs=1) as cst,
            tc.tile_pool(name="w", bufs=3) as wpool,
            tc.tile_pool(name="ut", bufs=2) as utpool,
            tc.tile_pool(name="d1", bufs=2) as d1pool,
            tc.tile_pool(name="cs", bufs=8) as cspool,
        ):
            # ---- resident inputs --------------------------------------------
            # A(0) only needs X1 (sync ring) + W0 (scalar ring); everything
            # else is emitted after A(0) so it doesn't steal DMA bandwidth
            # from the critical startup path.
            X1s = cst.tile([128, 4, B * L], F32R, tag="x1s")     # [i%128, it, b*256+x]
            nc.sync.dma_start(out=X1s[:], in_=X1T.rearrange("(it p) c -> p it c", p=128))
            X2s = cst.tile([128, 4, B * L], BF16, tag="x2s")     # [j%128, jt, b*256+y]
            D2C = [cst.tile([128, 2, O_LOC], F32, tag=f"d2c{b}", name=f"d2c{b}")
                   for b in range(B)]

            def emit_W_trigger(Ws, o2):
                nc.scalar.dma_start(
                    out=Ws[:],
                    in_=WM[2 * o2:2 * o2 + 2].rearrange("oi (it p) j -> p oi it j", p=128),
                )

            def emit_W(o2):
                Ws = wpool.tile([128, 2, 4, D], F32R, tag="ws")   # [i%128, oi, it, j]
                emit_W_trigger(Ws, o2)
                return Ws

            def emit_D1(o2, d1t=None):
                if d1t is None:
                    d1t = d1pool.tile([128, B * 512], F32, tag="d1t")
                nc.gpsimd.dma_start(out=d1t[:], in_=D1B[o2])
                return d1t

            # ---- main loop over o-pairs, software-pipelined ------------------
            ctx_psA = tc.tile_pool(name="psA", bufs=4, space="PSUM")
            ctx_psB = tc.tile_pool(name="psB", bufs=4, space="PSUM")
            psA = ctx_psA.__enter__()
            psB = ctx_psB.__enter__()

            def emit_A(o2, Ws):
                # step A: UT[bp][j, (jt, b2, oi, x)] for this o-pair; the
                # b2-major free layout keeps step B's rhs slice contiguous.
                UT = [utpool.tile([128, 4, 2, 2, 256], BF16, tag=f"utp{bp}", name=f"utp{bp}")
                      for bp in range(2)]
                for oi in range(2):
                    for jt in range(4):
                        for bp in range(2):
                            pa = psA.tile([128, 512], F32, tag="pa")
                            for it in range(4):
                                nc.tensor.matmul(
                                    pa[:],
                                    lhsT=Ws[:, oi, it, jt * 128:(jt + 1) * 128],
                                    rhs=X1s[:, it, bp * 512:(bp + 1) * 512],
                                    start=(it == 0), stop=(it == 3),
                                )
                            nc.vector.tensor_copy(
                                out=UT[bp][:, jt, :, oi, :], in_=pa[:])
                return UT

            def emit_B(o2, UT, d1t):
                # step B: out[y, (oi, x)] per (b, yt)
                for b in range(B):
                    bp, b2 = divmod(b, 2)
                    for yt in range(2):
                        pb = psB.tile([128, 512], F32, tag="pb")
                        for jt in range(4):
                            nc.tensor.matmul(
                                pb[:],
                                lhsT=X2s[:, jt, b * L + yt * 128: b * L + (yt + 1) * 128],
                                rhs=UT[bp][:, jt, b2, :, :],
                                start=(jt == 0), stop=(jt == 3),
                            )
                        cs = cspool.tile([128, 512], F32, tag="cs")
                        for oi in range(2):
                            og = 2 * o2 + oi
                            # cs = (pb + D2[y,o]) + D1[o,x], single DVE op
                            nc.vector.scalar_tensor_tensor(
                                out=cs[:, oi * 256:(oi + 1) * 256],
                                in0=pb[:, oi * 256:(oi + 1) * 256],
                                scalar=D2C[b][:, yt, og:og + 1],
                                in1=d1t[:, b * 512 + oi * 256: b * 512 + (oi + 1) * 256],
                                op0=ADD, op1=ADD,
                            )
                        nc.sync.dma_start(
                            out=OUT[b, 2 * o2:2 * o2 + 2, yt * 128:(yt + 1) * 128, :]
                                .rearrange("og y x -> y og x"),
                            in_=cs[:],
                        )

            # ---- startup gating ---------------------------------------------
            # Only X1 + W0 (4 MB) may be in flight at t=0: A(0) is gated on
            # exactly those.  Every other load is held back by a WAW edge on a
            # dummy write whose source (`gate`) reads X1s -- so those triggers
            # fire only once the X1 DMA completes, instead of stealing DMA
            # bandwidth from the critical startup path.
            W0 = emit_W(0)
            WsQ = [W0]
            lateW = [wpool.tile([128, 2, 4, D], F32R, tag="ws", name=f"wlate{i}")
                     for i in range(min(3, n_o2) - 1)]
            lateD1 = [d1pool.tile([128, B * 512], F32, tag="d1t", name=f"d1late{i}")
                      for i in range(min(2, n_o2))]
            gate = cst.tile([1, 8], F32R, tag="gate")
            nc.vector.tensor_copy(out=gate[:], in_=X1s[0:1, 0, 0:8])
            for t_ap in ([X2s[0:1, 0, 0:4]] +
                         [D2C[b][0:1, 0, 0:4] for b in range(B)] +
                         [w[0:1, 0, 0, 0:4] for w in lateW] +
                         [t[0:1, 0:4] for t in lateD1]):
                nc.vector.tensor_copy(out=t_ap, in_=gate[0:1, 0:4])
            for i, w in enumerate(lateW):
                emit_W_trigger(w, i + 1)
                WsQ.append(w)
            nc.sync.dma_start(out=X2s[:], in_=X2T.rearrange("(jt p) c -> p jt c", p=128))
            for b in range(B):
                nc.sync.dma_start(
                    out=D2C[b][:], in_=D2H[b].rearrange("p (yt o) -> p yt o", yt=2))
            D1Q = [emit_D1(i, t) for i, t in enumerate(lateD1)]

            UT_prev = emit_A(0, WsQ[0])
            for o2 in range(n_o2):
                if o2 + 3 < n_o2:
                    WsQ.append(emit_W(o2 + 3))
                if o2 + 2 < n_o2:
                    D1Q.append(emit_D1(o2 + 2))
                UT_next = emit_A(o2 + 1, WsQ[o2 + 1]) if o2 + 1 < n_o2 else None
                emit_B(o2, UT_prev, D1Q[o2])
                UT_prev = UT_next

            ctx_psB.__exit__(None, None, None)
            ctx_psA.__exit__(None, None, None)

    if split_waits:
        _split_multi_waits(nc)
    return nc


_NC_CACHE = None


def _get_nc():
    global _NC_CACHE
    if _NC_CACHE is None:
        _NC_CACHE = build_nc()
    return _NC_CACHE


def _prep_inputs(x1, x2, bw, W_bil, W_lin, b_lin):
    """Host-side glue: softmax of the 2-vector, per-core slicing/layout,
    and the cheap rank-2 D-terms (D1 = x1@G+g0, D2 = x2@V)."""
    x1 = np.asarray(x1, np.float32)
    x2 = np.asarray(x2, np.float32)
    bw = np.asarray(bw, np.float64)
    W_bil = np.asarray(W_bil, np.float32)
    W_lin = np.asarray(W_lin, np.float32)
    b_lin = np.asarray(b_lin, np.float32)

    e = np.exp(bw - bw.max())
    bwn = (e / e.sum()).astype(np.float32)
    bwn0, bwn1 = float(bwn[0]), float(bwn[1])

    x1T = np.ascontiguousarray(x1.transpose(2, 0, 1).reshape(D, B * L))
    x2T = np.ascontiguousarray(
        x2.transpose(2, 0, 1).reshape(D, B * L).astype(ml_dtypes.bfloat16))

    in_maps = []
    for c in range(N_CORES):
        o_sl = slice(c * O_LOC, (c + 1) * O_LOC)
        Wb = W_bil[o_sl]                                   # [16, 513, 513]
        WM = np.ascontiguousarray(bwn0 * Wb[:, :D, :D])
        G = bwn0 * Wb[:, :D, D].T + bwn1 * W_lin[:D, o_sl]     # [D, 16]
        V = bwn0 * Wb[:, D, :D].T + bwn1 * W_lin[D:, o_sl]     # [D, 16]
        g0 = bwn0 * Wb[:, D, D] + bwn1 * b_lin[o_sl]           # [16]

        D1 = np.einsum('bxd,do->bxo', x1, G) + g0              # [B, L, 16]
        D2 = np.einsum('byd,do->byo', x2, V)                   # [B, L, 16]
        # D1B[o2, p, (b, oi*256+x)] = D1[b, x, 2*o2+oi], replicated over p
        arr = D1.transpose(2, 0, 1).reshape(N_O2, 2, B, L)     # [o2, oi, b, x]
        arr = arr.transpose(0, 2, 1, 3).reshape(N_O2, 1, B * 512)
        D1Bv = np.ascontiguousarray(
            np.broadcast_to(arr, (N_O2, 128, B * 512)), dtype=np.float32)
        # D2H[b, p, (yt, o)] = D2[b, yt*128+p, o]
        D2Hv = np.ascontiguousarray(
            D2.reshape(B, 2, 128, O_LOC).transpose(0, 2, 1, 3)
            .reshape(B, 128, 2 * O_LOC), dtype=np.float32)

        in_maps.append({
            "WM": WM, "X1T": x1T, "X2T": x2T,
            "D1B": D1Bv, "D2H": D2Hv,
        })
    return in_maps


def _assemble(results):
    out = np.empty((B, L, L, O), np.float32)
    for c in range(N_CORES):
        # per-core OUT is [b, o_local, y, x] -> full is [b, x, y, o]
        out[:, :, :, c * O_LOC:(c + 1) * O_LOC] = \
            results[c]["OUT"].transpose(0, 3, 2, 1)
    return out


def kernel(**inputs):
    in_maps = _prep_inputs(**inputs)
    nc = _get_nc()
    res = run_bass_kernel_spmd(nc, in_maps, list(range(N_CORES)))
    return _assemble(res.results)


# revision 20
# speedup vs baseline: 1.4819x; 1.0640x over previous
"""Biaffine kernel for Trainium2, 8-core SPMD.

Math (reference):
    out[b,x,y,o] = bwn0 * sum_{i,j<=512} x1b[b,x,i] W_bil[o,i,j] x2b[b,y,j]
                 + bwn1 * (x1@W_lin[:512] [b,x,o] + x2@W_lin[512:] [b,y,o] + b_lin[o])
    with x1b/x2b = x append-ones, bwn = softmax(bw).

Decomposition (exact):
    out[b,x,y,o] = sum_{j<512} x2[b,y,j] * UT[b,o][j,x]      (step B, PE)
                 + D1[b,o,x] + D2[b,y,o]                      (fused DVE eviction)
    UT[b,o][j,x] = sum_{i<512} (bwn0*W_bil[o,i,j]) * x1[b,x,i]   (step A, PE)
    D1, D2 are cheap rank-2 terms (x1@G+g0, x2@V) computed on the HOST and
    shipped as inputs: D1 pre-replicated across the 128 partitions so the
    y-partitioned step-B eviction can read it directly, D2 in per-partition
    scalar layout.  This keeps the PE stream pure 512-row matmuls.

Sharding: tensor-parallel over O (128 output channels -> 16 per core).
Matmuls run as float32r (fp32 storage, TF32-like PE datapath, ~1e-4 rel err).

Pipelining: the PE p-state ramps 1.2->2.4 GHz only under continuous
execution, so emission is software-pipelined as A(0), [A(o2+1), B(o2)] --
step B of o2 runs right after step A of o2+1, hiding UT eviction latency.
W tiles prefetch 3 deep on the ACT-engine queue, D1 slabs 2 deep on the
GpSimd queue, so no trigger serializes behind the OUT-DMA waits on sync.
"""

import ml_dtypes
import numpy as np

import concourse.bass as bass
import concourse.mybir as mybir
import concourse.tile as tile
from concourse.bass_utils import run_bass_kernel_spmd

B, L, D, O = 4, 256, 512, 128
N_CORES = 8
O_LOC = O // N_CORES          # 16 output channels per core
N_O2 = O_LOC // 2             # 8 o-pairs per core
F32 = mybir.dt.float32
F32R = mybir.dt.float32r
BF16 = mybir.dt.bfloat16
ADD = mybir.AluOpType.add


# --------------------------------------------------------------------------
# Workaround: this container's walrus build accepts only ONE sync wait per
# instruction ("Too many sync wait commands").  Tile's wait assignment can
# attach several.  Post-pass: hoist extra waits onto InstEventSemaphore
# wait-carriers inserted immediately before the instruction on the same
# engine stream (same stall point, identical semantics).
_WS_CTR = [0]


def _split_multi_waits(nc):
    for f in nc.m.functions:
        for blk in f.blocks:
            insts = blk.instructions
            new = []
            changed = False
            for inst in insts:
                si = inst.sync_info
                waits = list(si.on_wait) if (si and si.on_wait) else []
                if len(waits) > 1:
                    for w in waits[:-1]:
                        _WS_CTR[0] += 1
                        carrier = mybir.InstEventSemaphore(
                            name=f"waitsplit_{_WS_CTR[0]}", ins=[], outs=[]
                        )
                        carrier.engine = inst.engine
                        carrier.sync_info = mybir.SyncInfo(on_wait=[w], on_update=[])
                        new.append(carrier)
                    si.on_wait = [waits[-1]]
                    changed = True
                new.append(inst)
            if changed:
                blk.instructions = new


# --------------------------------------------------------------------------
def build_nc(split_waits=True, n_o2=N_O2):
    nc = bass.Bass("TRN2", target_bir_lowering=False, debug=False,
                   num_devices=N_CORES)

    WM = nc.dram_tensor("WM", [O_LOC, D, D], BF16, kind="ExternalInput").ap()
    X1T = nc.dram_tensor("X1T", [D, B * L], BF16, kind="ExternalInput").ap()
    X2T = nc.dram_tensor("X2T", [D, B * L], BF16, kind="ExternalInput").ap()
    # D1 replicated across partitions: [o2, p, (b, oi*256+x)]
    D1B = nc.dram_tensor("D1B", [N_O2, 128, B * 512], F32, kind="ExternalInput").ap()
    # D2 in per-partition scalar layout: [b, p=y%128, (yt, o)]
    D2H = nc.dram_tensor("D2H", [B, 128, 2 * O_LOC], F32, kind="ExternalInput").ap()
    OUT = nc.dram_tensor("OUT", [B, O_LOC, L, L], F32, kind="ExternalOutput").ap()

    with tile.TileContext(nc) as tc:
        with (
            tc.tile_pool(name="const", bufs=1) as cst,
            tc.tile_pool(name="w", bufs=3) as wpool,
            tc.tile_pool(name="ut", bufs=2) as utpool,
            tc.tile_pool(name="d1", bufs=2) as d1pool,
            tc.tile_pool(name="cs", bufs=8) as cspool,
        ):
            # ---- resident inputs --------------------------------------------
            # A(0) only needs X1 (sync ring) + W0 (scalar ring); everything
            # else is emitted after A(0) so it doesn't steal DMA bandwidth
            # from the critical startup path.
            X1s = cst.tile([128, 4, B * L], BF16, tag="x1s")     # [i%128, it, b*256+x]
            nc.sync.dma_start(out=X1s[:], in_=X1T.rearrange("(it p) c -> p it c", p=128))
            X2s = cst.tile([128, 4, B * L], BF16, tag="x2s")     # [j%128, jt, b*256+y]
            D2C = [cst.tile([128, 2, O_LOC], F32, tag=f"d2c{b}", name=f"d2c{b}")
                   for b in range(B)]

            def emit_W_trigger(Ws, o2):
                nc.scalar.dma_start(
                    out=Ws[:],
                    in_=WM[2 * o2:2 * o2 + 2].rearrange("oi (it p) j -> p oi it j", p=128),
                )

            def emit_W(o2):
                Ws = wpool.tile([128, 2, 4, D], BF16, tag="ws")   # [i%128, oi, it, j]
                emit_W_trigger(Ws, o2)
                return Ws

            def emit_D1(o2, d1t=None):
                if d1t is None:
                    d1t = d1pool.tile([128, B * 512], F32, tag="d1t")
                nc.gpsimd.dma_start(out=d1t[:], in_=D1B[o2])
                return d1t

            # ---- main loop over o-pairs, software-pipelined ------------------
            ctx_psA = tc.tile_pool(name="psA", bufs=4, space="PSUM")
            ctx_psB = tc.tile_pool(name="psB", bufs=4, space="PSUM")
            psA = ctx_psA.__enter__()
            psB = ctx_psB.__enter__()

            def emit_A(o2, Ws):
                # step A: UT[bp][j, (jt, b2, oi, x)] for this o-pair; the
                # b2-major free layout keeps step B's rhs slice contiguous.
                UT = [utpool.tile([128, 4, 2, 2, 256], BF16, tag=f"utp{bp}", name=f"utp{bp}")
                      for bp in range(2)]
                for oi in range(2):
                    for jt in range(4):
                        for bp in range(2):
                            pa = psA.tile([128, 512], F32, tag="pa")
                            for it in range(4):
                                nc.tensor.matmul(
                                    pa[:],
                                    lhsT=Ws[:, oi, it, jt * 128:(jt + 1) * 128],
                                    rhs=X1s[:, it, bp * 512:(bp + 1) * 512],
                                    start=(it == 0), stop=(it == 3),
                                )
                            nc.vector.tensor_copy(
                                out=UT[bp][:, jt, :, oi, :], in_=pa[:])
                return UT

            def emit_B(o2, UT, d1t):
                # step B: out[y, (oi, x)] per (b, yt)
                for b in range(B):
                    bp, b2 = divmod(b, 2)
                    for yt in range(2):
                        pb = psB.tile([128, 512], F32, tag="pb")
                        for jt in range(4):
                            nc.tensor.matmul(
                                pb[:],
                                lhsT=X2s[:, jt, b * L + yt * 128: b * L + (yt + 1) * 128],
                                rhs=UT[bp][:, jt, b2, :, :],
                                start=(jt == 0), stop=(jt == 3),
                            )
                        cs = cspool.tile([128, 512], F32, tag="cs")
                        for oi in range(2):
                            og = 2 * o2 + oi
                            # cs = (pb + D2[y,o]) + D1[o,x], single DVE op
                            nc.vector.scalar_tensor_tensor(
                                out=cs[:, oi * 256:(oi + 1) * 256],
                                in0=pb[:, oi * 256:(oi + 1) * 256],
                                scalar=D2C[b][:, yt, og:og + 1],
                                in1=d1t[:, b * 512 + oi * 256: b * 512 + (oi + 1) * 256],
                                op0=ADD, op1=ADD,
                            )
                        nc.sync.dma_start(
                            out=OUT[b, 2 * o2:2 * o2 + 2, yt * 128:(yt + 1) * 128, :]
                                .rearrange("og y x -> y og x"),
                            in_=cs[:],
                        )

            # ---- startup gating ---------------------------------------------
            # Only X1 + W0 (4 MB) may be in flight at t=0: A(0) is gated on
            # exactly those.  Every other load is held back by a WAW edge on a
            # dummy write whose source (`gate`) reads X1s -- so those triggers
            # fire only once the X1 DMA completes, instead of stealing DMA
            # bandwidth from the critical startup path.
            W0 = emit_W(0)
            WsQ = [W0]
            lateW = [wpool.tile([128, 2, 4, D], BF16, tag="ws", name=f"wlate{i}")
                     for i in range(min(3, n_o2) - 1)]
            lateD1 = [d1pool.tile([128, B * 512], F32, tag="d1t", name=f"d1late{i}")
                      for i in range(min(2, n_o2))]
            gate = cst.tile([1, 8], BF16, tag="gate")
            nc.vector.tensor_copy(out=gate[:], in_=X1s[0:1, 0, 0:8])
            for t_ap in ([X2s[0:1, 0, 0:4]] +
                         [D2C[b][0:1, 0, 0:4] for b in range(B)] +
                         [w[0:1, 0, 0, 0:4] for w in lateW] +
                         [t[0:1, 0:4] for t in lateD1]):
                nc.vector.tensor_copy(out=t_ap, in_=gate[0:1, 0:4])
            for i, w in enumerate(lateW):
                emit_W_trigger(w, i + 1)
                WsQ.append(w)
            nc.sync.dma_start(out=X2s[:], in_=X2T.rearrange("(jt p) c -> p jt c", p=128))
            for b in range(B):
                nc.sync.dma_start(
                    out=D2C[b][:], in_=D2H[b].rearrange("p (yt o) -> p yt o", yt=2))
            D1Q = [emit_D1(i, t) for i, t in enumerate(lateD1)]

            UT_prev = emit_A(0, WsQ[0])
            for o2 in range(n_o2):
                if o2 + 3 < n_o2:
                    WsQ.append(emit_W(o2 + 3))
                if o2 + 2 < n_o2:
                    D1Q.append(emit_D1(o2 + 2))
                UT_next = emit_A(o2 + 1, WsQ[o2 + 1]) if o2 + 1 < n_o2 else None
                emit_B(o2, UT_prev, D1Q[o2])
                UT_prev = UT_next

            ctx_psB.__exit__(None, None, None)
            ctx_psA.__exit__(None, None, None)

    if split_waits:
        _split_multi_waits(nc)
    return nc


_NC_CACHE = None


def _get_nc():
    global _NC_CACHE
    if _NC_CACHE is None:
        _NC_CACHE = build_nc()
    return _NC_CACHE


def _prep_inputs(x1, x2, bw, W_bil, W_lin, b_lin):
    """Host-side glue: softmax of the 2-vector, per-core slicing/layout,
    and the cheap rank-2 D-terms (D1 = x1@G+g0, D2 = x2@V)."""
    x1 = np.asarray(x1, np.float32)
    x2 = np.asarray(x2, np.float32)
    bw = np.asarray(bw, np.float64)
    W_bil = np.asarray(W_bil, np.float32)
    W_lin = np.asarray(W_lin, np.float32)
    b_lin = np.asarray(b_lin, np.float32)

    e = np.exp(bw - bw.max())
    bwn = (e / e.sum()).astype(np.float32)
    bwn0, bwn1 = float(bwn[0]), float(bwn[1])

    x1T = np.ascontiguousarray(
        x1.transpose(2, 0, 1).reshape(D, B * L).astype(ml_dtypes.bfloat16))
    x2T = np.ascontiguousarray(
        x2.transpose(2, 0, 1).reshape(D, B * L).astype(ml_dtypes.bfloat16))

    in_maps = []
    for c in range(N_CORES):
        o_sl = slice(c * O_LOC, (c + 1) * O_LOC)
        Wb = W_bil[o_sl]                                   # [16, 513, 513]
        WM = np.ascontiguousarray((bwn0 * Wb[:, :D, :D]).astype(ml_dtypes.bfloat16))
        G = bwn0 * Wb[:, :D, D].T + bwn1 * W_lin[:D, o_sl]     # [D, 16]
        V = bwn0 * Wb[:, D, :D].T + bwn1 * W_lin[D:, o_sl]     # [D, 16]
        g0 = bwn0 * Wb[:, D, D] + bwn1 * b_lin[o_sl]           # [16]

        D1 = np.einsum('bxd,do->bxo', x1, G) + g0              # [B, L, 16]
        D2 = np.einsum('byd,do->byo', x2, V)                   # [B, L, 16]
        # D1B[o2, p, (b, oi*256+x)] = D1[b, x, 2*o2+oi], replicated over p
        arr = D1.transpose(2, 0, 1).reshape(N_O2, 2, B, L)     # [o2, oi, b, x]
        arr = arr.transpose(0, 2, 1, 3).reshape(N_O2, 1, B * 512)
        D1Bv = np.ascontiguousarray(
            np.broadcast_to(arr, (N_O2, 128, B * 512)), dtype=np.float32)
        # D2H[b, p, (yt, o)] = D2[b, yt*128+p, o]
        D2Hv = np.ascontiguousarray(
            D2.reshape(B, 2, 128, O_LOC).transpose(0, 2, 1, 3)
            .reshape(B, 128, 2 * O_LOC), dtype=np.float32)

        in_maps.append({
            "WM": WM, "X1T": x1T, "X2T": x2T,
            "D1B": D1Bv, "D2H": D2Hv,
        })
    return in_maps


def _assemble(results):
    out = np.empty((B, L, L, O), np.float32)
    for c in range(N_CORES):
        # per-core OUT is [b, o_local, y, x] -> full is [b, x, y, o]
        out[:, :, :, c * O_LOC:(c + 1) * O_LOC] = \
            results[c]["OUT"].transpose(0, 3, 2, 1)
    return out


def kernel(**inputs):
    in_maps = _prep_inputs(**inputs)
    nc = _get_nc()
    res = run_bass_kernel_spmd(nc, in_maps, list(range(N_CORES)))
    return _assemble(res.results)
